# revision 55
# baseline (speedup 1.0000x reference)
"""DocRE model kernel for 8 Trainium2 NeuronCores.

Data-parallel over the pair grid: core = b*4 + ib owns document b and
i-rows [8*ib, 8*ib+8) of the 32x32 entity-pair grid (256 pairs/core).
All weights are replicated; W_ext (49152x768) is streamed from HBM
through a fp16 matmul with the group-bilinear feature tiles
materialized on-chip.

v2: broadcast matmuls pipelined 2 kt-pairs ahead (PSUM: 4 feat banks +
4 rotating aux banks), cls projections folded into host-side biases,
PE clock warmup, prologue DMAs split across queues, early wc prefetch.
"""

import numpy as np

import concourse.bacc as bacc
import concourse.bass as bass
import concourse.tile as tile
from concourse import mybir
from concourse.bass_utils import run_bass_kernel_spmd
from concourse.masks import make_identity

F32 = mybir.dt.float32
F16 = mybir.dt.float16

B, L, H = 2, 1024, 768
E, M = 32, 4
EMB, BLK, NL = 768, 64, 97
G = EMB // BLK  # 12
LN_EPS = 1e-12

N_CORES = 8
IB = E // (N_CORES // B)     # 8 i-rows per core
NPAIR = IB * E               # 256 pairs per core
PT = NPAIR // 128            # 2 pair-tiles
KT = EMB * BLK // 128        # 384 k-tiles
KTP = KT // 2                # 192 kt-pairs
CT = EMB // 128              # 6 feature chunks
KC = H // 128                # 6 contraction chunks of H
LCOMP = 256                  # host-compacted mention positions
LC = LCOMP // 128            # 2 chunks of compacted length
NENT = IB + E                # 40 cols: [my 8 entities | all 32]
NE2 = NENT + 1


def _build_module():
    nc = bacc.Bacc("TRN2", target_bir_lowering=False, debug=False)

    seq_d = nc.dram_tensor("seq", [LCOMP, H], F16, kind="ExternalInput")
    S_d = nc.dram_tensor("S", [LCOMP, NENT], F16, kind="ExternalInput")
    Wh_d = nc.dram_tensor("Wh", [2 * H, EMB], F16, kind="ExternalInput")
    Wt_d = nc.dram_tensor("Wt", [2 * H, EMB], F16, kind="ExternalInput")
    bh_d = nc.dram_tensor("bh", [128, CT], F32, kind="ExternalInput")
    bt_d = nc.dram_tensor("bt", [128, G], F32, kind="ExternalInput")
    Wx_d = nc.dram_tensor("Wx", [EMB * BLK, EMB], F16, kind="ExternalInput")
    Ebc_d = nc.dram_tensor("Ebc", [128, 128], F16, kind="ExternalInput")
    bx_d = nc.dram_tensor("bx", [128, EMB], F32, kind="ExternalInput")
    blog_d = nc.dram_tensor("blog", [128, NL], F32, kind="ExternalInput")
    Wc_d = nc.dram_tensor("Wc", [EMB, NL], F32, kind="ExternalInput")
    out_d = nc.dram_tensor("out", [NPAIR, NL], F32, kind="ExternalOutput")

    with tile.TileContext(nc) as tc:
        with (
            tc.tile_pool(name="persist", bufs=1) as persist,
            tc.tile_pool(name="seqp", bufs=1) as seqp,
            tc.tile_pool(name="whp", bufs=24) as whp,
            tc.tile_pool(name="wxp", bufs=12) as wxp,
            tc.tile_pool(name="blp", bufs=16) as blp,
            tc.tile_pool(name="tmpp", bufs=3) as tmpp,
            tc.tile_pool(name="dramp", bufs=1, space="DRAM") as dramp,
            tc.tile_pool(name="psf", bufs=1, space="PSUM") as psf,
            tc.tile_pool(name="aux", bufs=4, space="PSUM") as auxp,
        ):
            def aux_tile(cols=512):
                return auxp.tile([128, cols], F32, name="aux", tag="aux",
                                 padded_shape=[128, 512])

            ident = persist.tile([128, 128], F32, name="ident")
            make_identity(nc, ident[:])
            # E selector padded to a full 128x128 tile so the bc matmul
            # uses the same PE tile config as the main matmuls (avoids
            # the tile-size reconfig stall seen with a [2,128] operand).
            E_t = persist.tile([128, 128], F16, name="E_t")
            nc.sync.dma_start(E_t[:], Ebc_d.ap())

            # hs staging ping-pong buffers; rows 2-127 stay zero so the
            # padded-K bc contraction adds exact zeros.
            hsgA = persist.tile([128, 16, 512], F16, name="hsgA")
            hsgB = persist.tile([128, 16, 512], F16, name="hsgB")
            # zero only rows 2-64: the bc matmul contracts K=65 (same
            # 128-row PE tile config) so higher rows are never read.
            # scalar memzero: Copy is in every act table set, and the
            # scalar queue is idle this early.
            nc.scalar.memzero(hsgA[:])
            nc.scalar.memzero(hsgB[:])

            # ---- PE clock warmup: dummy transposes to ramp the tensor
            # engine to full p-state while input DMAs land.
            for _ in range(3):
                wps = aux_tile()
                nc.tensor.transpose(wps[:, 0:128], ident[:], ident[:])
                nc.tensor.transpose(wps[:, 128:256], ident[:], ident[:])
                nc.tensor.transpose(wps[:, 256:384], ident[:], ident[:])
                nc.tensor.transpose(wps[:, 384:512], ident[:], ident[:])

            # ---- per-column constants broadcast to all partitions ----
            bx_b = persist.tile([128, EMB], F32, name="bx_b")
            blog_b = persist.tile([128, NL], F32, name="blog_b")
            for tile_, src in ((bx_b, bx_d), (blog_b, blog_d)):
                nc.scalar.dma_start(tile_[:], src.ap())

            eps_t = persist.tile([128, 1], F32, name="eps")
            nc.vector.memset(eps_t[:], LN_EPS)

            # prewarm the Tanh activation table while initial DMAs land
            # so the first hs/ts activation pays no ACT_TABLE_LOAD.
            warm_tanh = tmpp.tile([1, 1], F32, name="warm_tanh")
            nc.scalar.activation(warm_tanh[:], ident[0:1, 0:1],
                                 mybir.ActivationFunctionType.Tanh,
                                 bias=0.0, scale=1.0)

            # tanh bias chunks (b + cls@W[2H:3H] folded in host-side);
            # bt is in 64-row-duplicated layout matching tsdup partitions
            bh_t = persist.tile([128, CT], F32, name="bh_t")
            bt_t = persist.tile([128, G], F32, name="bt_t")
            for tile_, src in ((bh_t, bh_d), (bt_t, bt_d)):
                nc.scalar.dma_start(tile_[:], src.ap())

            # classifier weights (ln scale pre-folded) prefetched early
            wc_t = persist.tile([128, CT, NL], F32, name="wc_t")
            nc.scalar.dma_start(wc_t[:], Wc_d.ap().rearrange("(c p) n -> p c n", p=128))

            # ---- phase E: entity pooling  ent = S^T @ seq ----
            seq_t = seqp.tile([128, LC, H], F16, name="seq_t")
            S_t = seqp.tile([128, LC, NENT], F16, name="S_t")
            seq_re = seq_d.ap().rearrange("(c p) h -> p c h", p=128)
            S_re = S_d.ap().rearrange("(c p) n -> p c n", p=128)
            for kc in range(LC):
                nc.sync.dma_start(S_t[:, kc, :], S_re[:, kc, :])
                nc.sync.dma_start(seq_t[:, kc, :], seq_re[:, kc, :])

            # preload all phase-A weight tiles on the sync queue AFTER
            # the (compacted) phase-E loads — queue order gives phase E
            # strict DMA priority, then the chains stream in weights.
            wts = {}
            for m, (w_d, blk) in enumerate(((Wh_d, 0), (Wh_d, 1),
                                            (Wt_d, 0), (Wt_d, 1))):
                for kc in range(KC):
                    w_t = whp.tile([128, EMB], F16, name="w_t")
                    nc.sync.dma_start(
                        w_t[:],
                        w_d.ap()[blk * H + kc * 128: blk * H + (kc + 1) * 128, :])
                    wts[(m, kc)] = w_t

            ps_e0 = aux_tile()
            ps_e1 = aux_tile(256)
            for kc in range(LC):
                nc.tensor.matmul(ps_e0[:NENT, :], S_t[:, kc, :], seq_t[:, kc, 0:512],
                                 start=(kc == 0), stop=(kc == LC - 1))
                nc.tensor.matmul(ps_e1[:NENT, :], S_t[:, kc, :], seq_t[:, kc, 512:768],
                                 start=(kc == 0), stop=(kc == LC - 1))
            ent_nat = persist.tile([NENT, H], F32, name="ent_nat")
            nc.vector.tensor_copy(ent_nat[:, 0:512], ps_e0[:NENT, :])
            nc.vector.tensor_copy(ent_nat[:, 512:768], ps_e1[:NENT, :])

            # transpose ent -> entT [h, NENT]
            entT = persist.tile([128, KC, NENT], F16, name="entT")
            for kc in range(KC):
                ps_tr = aux_tile(NENT)
                nc.tensor.transpose(ps_tr[:, :NENT], ent_nat[:, kc * 128:(kc + 1) * 128],
                                    ident[:NENT, :NENT])
                nc.vector.tensor_copy(entT[:, kc, :], ps_tr[:, :NENT])

            # ---- phase A: A/B projections (head first: its downstream
            # path through hs_dram/staging is the longest) ----
            ABCD = []
            for ct in range(CT):
                abcd_alloc = persist.tile([128, 4, NE2], F32, name=f"abcd{ct}")
                nc.vector.memset(abcd_alloc[:], 0.0)
                ABCD.append(abcd_alloc)

            ps_feat = [[psf.tile([128, 512], F32, name=f"pf{pt}a"),
                        psf.tile([128, 256], F32, name=f"pf{pt}b")]
                       for pt in range(PT)]

            def emit_ab_chain(m, ps_pair):
                ps_n0 = ps_pair[0][:NENT, :]
                ps_n1 = ps_pair[1][:NENT, :]
                for kc in range(KC):
                    w_t = wts[(m, kc)]
                    nc.tensor.matmul(ps_n0[:], entT[:, kc, :], w_t[:, 0:512],
                                     start=(kc == 0), stop=(kc == KC - 1))
                    nc.tensor.matmul(ps_n1[:], entT[:, kc, :], w_t[:, 512:768],
                                     start=(kc == 0), stop=(kc == KC - 1))
                x_nat = tmpp.tile([NENT, EMB], F32, name="x_nat")
                nc.vector.tensor_copy(x_nat[:, 0:512], ps_n0[:])
                nc.vector.tensor_copy(x_nat[:, 512:768], ps_n1[:])
                for ct in range(CT):
                    ps_tr = aux_tile(NENT)
                    nc.tensor.transpose(ps_tr[:, :NENT], x_nat[:, ct * 128:(ct + 1) * 128],
                                        ident[:NENT, :NENT])
                    nc.vector.tensor_copy(ABCD[ct][:, m, 0:NENT], ps_tr[:, :NENT])

            def colview(tile_, m, col0, ap_pat):
                return bass.AP(tensor=tile_.tensor,
                               offset=tile_.offset + m * NE2 + col0,
                               ap=[tile_.ap[0]] + ap_pat)

            # ---- head side: projections, hs generation, DRAM staging ----
            emit_ab_chain(0, ps_feat[0])
            emit_ab_chain(1, ps_feat[1])

            hsT = persist.tile([128, CT, 2 * 128], F16, name="hsT")
            hs_dram = dramp.tile([EMB, 2 * 128], F16, name="hs_dram")
            for ct in range(CT):
                abcd_t = ABCD[ct]
                tmp = tmpp.tile([128, 8, 32], F32, name="tmp")
                nc.vector.tensor_tensor(
                    tmp[:], colview(abcd_t, 0, 0, [[1, 8], [0, 32]]),
                    colview(abcd_t, 1, IB, [[0, 8], [1, 32]]),
                    op=mybir.AluOpType.add)
                nc.scalar.activation(
                    hsT[:, ct, :].rearrange("p (a b) -> p a b", a=8),
                    tmp[:], mybir.ActivationFunctionType.Tanh,
                    bias=bh_t[:, ct:ct + 1], scale=1.0)
                nc.sync.dma_start(hs_dram[ct * 128:(ct + 1) * 128, :],
                                  hsT[:, ct, :])

            def stage_group(g):
                hsg_tiles[g] = hsgA if g % 2 == 0 else hsgB
                nc.scalar.dma_start(
                    hsg_tiles[g][0:2, :, :].rearrange("r tq (q p) -> r tq q p", q=2),
                    bass.AP(tensor=hs_dram.tensor,
                            offset=hs_dram.offset + g * 64 * 2 * 128,
                            ap=[[256, 2], [4 * 256, 16], [2 * 256, 2], [1, 256]]))

            hsg_tiles = {}
            stage_group(0)

            # ---- tail side: projections, tsdup generation ----
            emit_ab_chain(2, ps_feat[0])
            emit_ab_chain(3, ps_feat[1])

            # compute each group's ts half on its native 64 partitions,
            # then replicate the finished fp16 result with one DMA —
            # halves the scalar-engine serial work vs duplicating first.
            tsdup = persist.tile([128, G, 2 * 128], F16, name="tsdup")

            bc_tiles = {}
            bl_tiles = {}

            def emit_bc_pair(ktp):
                kt0 = 2 * ktp
                g, tq = kt0 // 32, (kt0 % 32) // 2
                if kt0 % 32 == 16 and g + 1 < G:
                    stage_group(g + 1)
                bc_ps = aux_tile()
                nc.tensor.matmul(bc_ps[:], E_t[0:65, :], hsg_tiles[g][0:65, tq, :],
                                 start=True, stop=True)
                bc_tiles[ktp] = bc_ps

            def emit_tt_pair(ktp):
                bc_ps = bc_tiles[ktp]
                for q in range(2):
                    kt = 2 * ktp + q
                    g = kt // 32
                    bl_t = blp.tile([128, 2 * 128], F16, name="bl_t")
                    nc.vector.tensor_tensor(bl_t[:], bc_ps[:, q * 256:(q + 1) * 256],
                                            tsdup[:, g, :],
                                            op=mybir.AluOpType.mult)
                    bl_tiles[kt] = bl_t

            for ct in range(CT):
                abcd_t = ABCD[ct]
                for half in range(2):
                    g = 2 * ct + half
                    p0, p1 = half * 64, half * 64 + 64
                    sub = abcd_t[p0:p1, :, :]
                    tmp2 = tmpp.tile([128, 8, 32], F32, name="tmp")
                    nc.vector.tensor_tensor(
                        tmp2[p0:p1], colview(sub, 2, IB, [[0, 8], [1, 32]]),
                        colview(sub, 3, 0, [[1, 8], [0, 32]]),
                        op=mybir.AluOpType.add)
                    nc.scalar.activation(
                        tsdup[p0:p1, g, :].rearrange("p (a b) -> p a b", a=8),
                        tmp2[p0:p1], mybir.ActivationFunctionType.Tanh,
                        bias=bt_t[p0:p1, g:g + 1], scale=1.0)
                    dmae = nc.sync if half == 0 else nc.scalar
                    dmae.dma_start(tsdup[128 - p1:128 - p0, g, :],
                                   tsdup[p0:p1, g, :])

            # prewarm the Relu/Sqrt activation table while the scalar
            # engine is idle so phase L pays no ACT_TABLE_LOAD.
            # the input dep on the last tsdup tile pins this after all
            # tanhs in the schedule, so the sqrt-set table load lands in
            # main-loop scalar idle time instead of inside phase L.
            warm_act = tmpp.tile([1, 1], F32, name="warm_act")
            nc.scalar.activation(warm_act[:], tsdup[64:65, G - 1, 0:1],
                                 mybir.ActivationFunctionType.Sqrt,
                                 bias=eps_t[0:1, :], scale=1.0)

            # ---- phase M: main contraction over W_ext ----
            # bc broadcasts run 2 kt-pairs ahead of the consuming matmuls
            # so the DVE bl multiply is never on the PE critical path.
            def emit_main_mms(kt, wx_t, bl_t, pts):
                for pt in pts:
                    lhsT = bl_t[:, pt * 128:(pt + 1) * 128]
                    nc.tensor.matmul(ps_feat[pt][0][:], lhsT, wx_t[:, 0:512],
                                     start=(kt == 0), stop=(kt == KT - 1))
                    nc.tensor.matmul(ps_feat[pt][1][:], lhsT, wx_t[:, 512:768],
                                     start=(kt == 0), stop=(kt == KT - 1))

            # last STAG k-tiles run pt0-only first, then pt1 — pt0's
            # accumulators stop ~4us early so its layernorm chain
            # overlaps the final pt1 matmuls.
            STAG = 12
            tail = {}
            emit_bc_pair(0)
            emit_bc_pair(1)
            emit_tt_pair(0)
            for ktp in range(KTP):
                if ktp + 2 < KTP:
                    emit_bc_pair(ktp + 2)
                if ktp + 1 < KTP:
                    emit_tt_pair(ktp + 1)
                for q in range(2):
                    kt = 2 * ktp + q
                    wx_t = wxp.tile([128, EMB], F16, name="wx_t")
                    nc.sync.dma_start(wx_t[:], Wx_d.ap()[kt * 128:(kt + 1) * 128, :])
                    bl_t = bl_tiles.pop(kt)
                    if kt < KT - STAG:
                        emit_main_mms(kt, wx_t, bl_t, range(PT))
                    else:
                        emit_main_mms(kt, wx_t, bl_t, [0])
                        tail[kt] = (wx_t, bl_t)
                bc_tiles.pop(ktp)
            for kt in sorted(tail):
                wx_t, bl_t = tail[kt]
                emit_main_mms(kt, wx_t, bl_t, [1])

            # ---- phase L: bias, relu, layernorm, classifier ----
            for pt in range(PT):
                feat = persist.tile([128, EMB], F32, name=f"feat{pt}")
                nc.vector.tensor_tensor(feat[:, 0:512], ps_feat[pt][0][:],
                                        bx_b[:, 0:512], op=mybir.AluOpType.add)
                nc.vector.tensor_tensor(feat[:, 512:768], ps_feat[pt][1][:],
                                        bx_b[:, 512:768], op=mybir.AluOpType.add)
                nc.scalar.activation(feat[:], feat[:],
                                     mybir.ActivationFunctionType.Relu,
                                     bias=0.0, scale=1.0)

                stats = tmpp.tile([128, 3, 6], F32, name="stats")
                f_re = feat.rearrange("p (c f) -> p c f", c=3)
                for c in range(3):
                    nc.vector.bn_stats(stats[:, c, :], f_re[:, c, :])
                mv = tmpp.tile([128, 2], F32, name="mv")
                nc.vector.bn_aggr(mv[:], stats[:])
                sd = tmpp.tile([128, 1], F32, name="sd")
                nc.scalar.activation(sd[:], mv[:, 1:2],
                                     mybir.ActivationFunctionType.Sqrt,
                                     bias=eps_t[:], scale=1.0)
                rstd = tmpp.tile([128, 1], F32, name="rstd")
                nc.vector.reciprocal(rstd[:], sd[:])

                # normalized activations; ln_g/ln_b are folded into
                # Wc / blog host-side.
                ln = persist.tile([128, EMB], F32, name=f"ln{pt}")
                nc.vector.tensor_scalar(ln[:], feat[:], mv[:, 0:1], rstd[:],
                                        op0=mybir.AluOpType.subtract,
                                        op1=mybir.AluOpType.mult)

                lnT = persist.tile([128, CT, 128], F32, name=f"lnT{pt}")
                for ct in range(CT):
                    ps_tr2 = aux_tile(128)
                    nc.tensor.transpose(ps_tr2[:, 0:128], ln[:, ct * 128:(ct + 1) * 128],
                                        ident[:])
                    nc.scalar.copy(lnT[:, ct, :], ps_tr2[:, 0:128])

                ps_lg = aux_tile(NL)
                for ct in range(CT):
                    nc.tensor.matmul(ps_lg[:, :NL], lnT[:, ct, :], wc_t[:, ct, :],
                                     start=(ct == 0), stop=(ct == CT - 1))
                out_sb = tmpp.tile([128, NL], F32, name="out_sb")
                nc.vector.tensor_tensor(out_sb[:], ps_lg[:, :NL], blog_b[:],
                                        op=mybir.AluOpType.add)
                nc.scalar.dma_start(out_d.ap()[pt * 128:(pt + 1) * 128, :], out_sb[:])

    nc.compile()
    return nc


_NC_CACHE = []


def _get_module():
    if not _NC_CACHE:
        _NC_CACHE.append(_build_module())
    return _NC_CACHE[0]


_EBC = np.zeros((128, 128), np.float16)
_EBC[0, :64] = 1.0
_EBC[1, 64:] = 1.0


def _build_inputs(seq, starts, ends, mention_mask, W_head, b_head, W_tail, b_tail,
                  W_ext, b_ext, ln_g, ln_b, W_cls):
    seq = np.asarray(seq, np.float32)
    starts = np.asarray(starts, np.int64)
    ends = np.asarray(ends, np.int64)
    mask = np.asarray(mention_mask, np.float32)
    W_head = np.asarray(W_head, np.float32)
    W_tail = np.asarray(W_tail, np.float32)

    # per-document entity selection matrix: ent = Sb^T @ seq[b]
    S_b = np.zeros((B, L, E), np.float32)
    denom = np.maximum(mask.sum(axis=2), 1.0)          # [B, E]
    w = mask * 0.5 / denom[:, :, None]                 # [B, E, M]
    for b in range(B):
        for e in range(E):
            np.add.at(S_b[b, :, e], starts[b, e] + 1, w[b, e])
            np.add.at(S_b[b, :, e], ends[b, e], w[b, e])

    # cls projection folded into the tanh bias (host-side, per document)
    cls = seq[:, 0]                                    # [B, H]
    cbias_h = cls @ W_head[2 * H:] + np.asarray(b_head, np.float32)  # [B, EMB]
    cbias_t = cls @ W_tail[2 * H:] + np.asarray(b_tail, np.float32)

    ln_g = np.asarray(ln_g, np.float32)
    ln_b = np.asarray(ln_b, np.float32)
    W_cls = np.asarray(W_cls, np.float32)
    shared = {
        "Wh": np.ascontiguousarray(W_head[:2 * H].astype(np.float16)),
        "Wt": np.ascontiguousarray(W_tail[:2 * H].astype(np.float16)),
        "Wx": np.ascontiguousarray(np.asarray(W_ext).astype(np.float16)),
        "Ebc": _EBC,
        "bx": np.ascontiguousarray(np.broadcast_to(np.asarray(b_ext, np.float32), (128, EMB))),
        "blog": np.ascontiguousarray(np.broadcast_to(ln_b @ W_cls, (128, NL))),
        "Wc": np.ascontiguousarray(ln_g[:, None] * W_cls),
    }
    # compact the sequence to the <=256 positions the pooling matrix
    # actually selects (padding rows get zero weights, exact result)
    seq_c = np.zeros((B, LCOMP, H), np.float16)
    S_c = np.zeros((B, LCOMP, E), np.float32)
    for b in range(B):
        pos = np.unique(np.concatenate([starts[b].ravel() + 1, ends[b].ravel()]))
        npos = len(pos)
        seq_c[b, :npos] = seq[b][pos].astype(np.float16)
        S_c[b, :npos] = S_b[b][pos]

    in_maps = []
    for core in range(N_CORES):
        b, ib = core // 4, core % 4
        S_core = np.concatenate(
            [S_c[b][:, ib * IB:(ib + 1) * IB], S_c[b]], axis=1)
        cb = cbias_t[b].reshape(G, BLK)
        in_maps.append({
            "seq": np.ascontiguousarray(seq_c[b]),
            "S": np.ascontiguousarray(S_core.astype(np.float16)),
            "bh": np.ascontiguousarray(cbias_h[b].reshape(CT, 128).T),
            "bt": np.ascontiguousarray(np.vstack([cb.T, cb.T])),
            **shared,
        })
    return in_maps


def kernel(**inputs) -> np.ndarray:
    nc = _get_module()
    in_maps = _build_inputs(**inputs)
    res = run_bass_kernel_spmd(nc, in_maps, core_ids=list(range(N_CORES)))
    outs = np.stack([res.results[c]["out"] for c in range(N_CORES)])  # [8,256,97]
    return outs.reshape(B, 4, IB, E, NL).reshape(B, E, E, NL)
